# revision 1
# baseline (speedup 1.0000x reference)
"""Trainium2 Bass kernel for nn_BasicBlock_1w4a_LUT (binarized 3x3 conv + LUT bucketize).

Data-parallel over batch: 8 NeuronCores x 4 images each; no cross-core
communication. Full inputs in, full output out; shard/unshard on the host.

Host prep:
  - Binarize the weights exactly as the reference does; the result is
    sign(bw)*sw with sw>0 per out-channel. sw is folded into the LUT
    thresholds so the device weights are exactly +/-1 (exact in fp16).
  - Split x into hi/lo fp16 (x == hi + lo to ~2^-21 relative; the PE
    multiplies fp16 exactly, verified on HW) and zero-pad each image into a
    "flat" 114x114 slab: partitions 0-63 hi, 64-127 lo. The 3x3/pad-1 conv
    then becomes 9 shifted dot products over the flat vector; the 2 junk
    columns per row at the wrap seams are computed anyway and stripped on
    the host.
  - Per-channel affine z = y*s + b chosen so bucketize thresholds map to
    tau3 -> 0 and tau5 -> 1 (frees DVE scalar slots; the DVE op set below
    only has 3 per-partition scalar slots per instruction).

Device, per image:
  - 9 accumulated K=128 fp16 matmuls per 456-pixel chunk (K packs hi+lo of
    one tap; weights duplicated across the two halves). Two chunks run
    concurrently on the PE via column tiling (tile_position), which also
    fills all 128 PSUM partitions for the vector ops. Runs at the PE
    streaming bound: 1 column/cycle/pair at 2.4 GHz.
  - ScalarE applies the per-channel affine out of PSUM; two custom DVE ops
    (registered at import time into concourse's custom-DVE table) compute
    out = sum_k [z > tau_k] over all 7 thresholds in 2 passes, writing u8.
  - A few junk matmuls at kernel start warm the PE HAM clock gate
    (1.2 -> 2.4 GHz) while the first input DMA pieces land; input/output
    DMAs are split so compute starts/finishes without waiting for whole
    images.
"""

import numpy as np

# ---- problem constants (hardcoded per contract) ----
B, Cin, Cout, H, W = 32, 64, 64, 112, 112
NCORES = 8
BPC = B // NCORES          # images per core
HP = H + 2                 # 114 padded rows
WPAD = W + 2               # 114 padded cols
FLAT = HP * WPAD           # 12996 padded image size
HALF = H * W // 2          # 6272 output pixels per column-tile half
NCH = 4 * W                # 448 pixels per chunk = 4 image rows (PSUM <= 2KB)
NPAIR = H // 8             # 14 adjacent chunk pairs per image
SLABF = HP * WPAD          # 12996: the whole padded image is the slab
NTAPS = 9
NSPLIT = 4                 # input slab DMA pieces per image (after the first)
NWARM = 13                 # PE warm-up matmuls

_built = []
last_results = None


def _register_dve_ops():
    from concourse.dve_spec import (
        Spec, Src0, Src1, C0, C1, C3, One, Zero, lower,
        _spill_c3_to_src1, _has_src1,
    )
    import concourse.dve_ops as dve_ops
    from concourse.dve_ops import DveOp
    from concourse.dve_uop import DveOpSpec

    def register_op(name, spec):
        if name in dve_ops._SUB_OPCODE_FOR_NAME:
            for op in dve_ops.OPS:
                if op.name == name:
                    return op
            raise RuntimeError(name)
        row = max(dve_ops._SUB_OPCODE_FOR_NAME.values()) + 1
        assert row < 0x20, "custom-DVE opcode rows exhausted"
        shas = {}
        for ver in ("v3", "v4"):
            s = DveOpSpec(name=name, opcode=row, uops=lower(spec, ver=ver),
                          rd1_en=_has_src1(spec))
            shas[ver] = s.sha(ver)
        op = DveOp(name, spec, subdim=False, uops_sha=shas)
        dve_ops.OPS.append(op)
        dve_ops.CUSTOM_DVE_SPECS[name] = spec
        dve_ops._SUB_OPCODE_FOR_NAME[name] = row
        return op

    # u = (z>tau0) + (z>tau1) + (z>tau2);  tau2 rides C3 (spilled to in1 [P,1])
    bucket3 = register_op(
        "BUCKET3_ANT",
        Spec(
            body=_spill_c3_to_src1(((Src0 > C0) + (Src0 > C1)) + (Src0 > C3)),
            reference=lambda in0, in1, s0, s1, imm2: (
                (in0 > s0).astype(np.float32) + (in0 > s1)
                + (in0 > np.asarray(in1, np.float32).reshape(-1, 1))
            ),
        ),
    )
    # out = (z>0) + (z>1) + (z>tau4) + (z>tau6) + u
    bucket4acc = register_op(
        "BUCKET4ACC_ANT",
        Spec(
            body=(((Src0 > Zero) + (Src0 > One))
                  + ((Src0 > C0) + (Src0 > C1))) + Src1,
            reference=lambda in0, in1, s0, s1, imm2: (
                (in0 > 0).astype(np.float32) + (in0 > 1)
                + (in0 > s0) + (in0 > s1) + in1
            ),
        ),
    )
    return bucket3, bucket4acc


def _build():
    """Trace + compile the per-core Bass kernel (once per process)."""
    if _built:
        return _built[0]

    import concourse.bacc as bacc
    import concourse.mybir as mybir
    import concourse.tile as tile

    bucket3, bucket4acc = _register_dve_ops()

    f32, f16, u8 = mybir.dt.float32, mybir.dt.float16, mybir.dt.uint8
    nc = bacc.Bacc("TRN2", target_bir_lowering=False, debug=False,
                   num_devices=NCORES)

    xin_t = nc.dram_tensor("xin", [BPC, 128, SLABF], f16, kind="ExternalInput")
    wts_t = nc.dram_tensor("wts", [128, NTAPS, Cout], f16, kind="ExternalInput")
    nrm_t = nc.dram_tensor("nrm", [128, 7], f32, kind="ExternalInput")
    out_t = nc.dram_tensor("out", [BPC, 128, HALF], u8, kind="ExternalOutput")

    with tile.TileContext(nc) as tc:
        with (
            tc.tile_pool(name="const", bufs=1) as cpool,
            tc.tile_pool(name="slab", bufs=2) as spool,
            tc.tile_pool(name="psum", bufs=6, space="PSUM") as ppool,
            tc.tile_pool(name="wpsum", bufs=1, space="PSUM") as wpool,
            tc.tile_pool(name="z", bufs=4) as zpool,
            tc.tile_pool(name="u", bufs=4) as upool,
            tc.tile_pool(name="o", bufs=2) as opool,
        ):
            # first slab piece of image 0 goes out before anything else — it
            # gates the first matmuls
            slab0 = spool.tile([128, SLABF], f16, tag="slab")
            nc.sync.dma_start(out=slab0[:, 0:1400], in_=xin_t.ap()[0, :, 0:1400])
            wts = cpool.tile([128, NTAPS, Cout], f16)
            nc.scalar.dma_start(out=wts[:], in_=wts_t.ap())
            nrm = cpool.tile([128, 7], f32)
            nc.scalar.dma_start(out=nrm[:], in_=nrm_t.ap())

            # PE warm-up: junk matmuls on a zeroed tile while the first input
            # DMAs land, so the HAM clock gate opens (1.2 -> 2.4 GHz) before
            # the real matmuls start.
            wu = cpool.tile([128, NCH], f16)
            nc.gpsimd.memset(wu[:], 0.0)
            wps = wpool.tile([64, NCH], f32)
            for _ in range(NWARM):
                nc.tensor.matmul(wps[:], wu[:, 0:Cout], wu[:],
                                 tile_position=(0, 0), start=True, stop=True)

            scale, bias = nrm[:, 0:1], nrm[:, 1:2]
            tau0, tau1, tau2 = nrm[:, 2:3], nrm[:, 3:4], nrm[:, 4:5]
            tau4, tau6 = nrm[:, 5:6], nrm[:, 6:7]

            for b in range(BPC):
                # split the input DMA so early chunks can start sooner;
                # finer-grained for the first image (it gates the pipeline)
                if b == 0:
                    slab = slab0
                    cuts = [1400, 3200, 5400, 8000, 10500, SLABF]
                else:
                    slab = spool.tile([128, SLABF], f16, tag="slab")
                    step = (SLABF + NSPLIT - 1) // NSPLIT
                    cuts = [min(k * step, SLABF) for k in range(NSPLIT + 1)]
                for n, (lo, hi) in enumerate(zip(cuts[:-1], cuts[1:])):
                    eng = nc.sync if n % 2 == 0 else nc.scalar
                    eng.dma_start(out=slab[:, lo:hi], in_=xin_t.ap()[b, :, lo:hi])
                slabv = slab[:].rearrange("p (r w) -> p r w", w=WPAD)

                oslab = opool.tile([128, HALF], u8)
                for j in range(NPAIR):
                    ps = ppool.tile([128, NCH], f32)
                    # the two column-tiled halves (adjacent 4-row chunks) are
                    # issued interleaved per tap so the PE runs them
                    # concurrently; the row-structured rhs AP (4 x 112 of the
                    # padded 114-wide slab) skips the seam columns entirely
                    for t in range(NTAPS):
                        dh, dw = divmod(t, 3)
                        for half in range(2):
                            r0 = 4 * (2 * j + half) + dh
                            nc.tensor.matmul(
                                ps[half * Cout:(half + 1) * Cout, :],
                                wts[:, t, :],
                                slabv[:, r0:r0 + 4, dw:dw + W],
                                tile_position=(0, half * Cout),
                                start=(t == 0), stop=(t == NTAPS - 1))
                    z = zpool.tile([128, NCH], f32)
                    nc.scalar.activation(
                        z[:], ps[:], mybir.ActivationFunctionType.Identity,
                        bias=bias, scale=scale)
                    u = upool.tile([128, NCH], mybir.dt.bfloat16)
                    nc.vector._custom_dve(
                        bucket3, out=u[:], in0=z[:],
                        in1=tau2, s0=tau0, s1=tau1)
                    nc.vector._custom_dve(
                        bucket4acc, out=oslab[:, j * NCH:(j + 1) * NCH],
                        in0=z[:], in1=u[:], s0=tau4, s1=tau6)

                # split output DMA so early pieces leave while later chunks
                # are still being computed; finest for the last image (tail)
                ocuts = ([0, HALF // 2, 3 * HALF // 4, 7 * HALF // 8, HALF]
                         if b == BPC - 1 else [0, HALF // 2, HALF])
                for lo, hi in zip(ocuts[:-1], ocuts[1:]):
                    nc.sync.dma_start(out=out_t.ap()[b, :, lo:hi],
                                      in_=oslab[:, lo:hi])

    nc.compile()
    _built.append(nc)
    return nc


def _binarize_weights(w):
    """Exactly the reference's fp32 binarization. Returns (sign in {-1,0,1}, sw)."""
    w = np.asarray(w, np.float32)
    C = w.shape[0]
    wf = w.reshape(C, -1)
    bw = w - wf.mean(-1)[:, None, None, None]
    bw = bw / bw.reshape(C, -1).std(-1, ddof=1)[:, None, None, None]
    mean_abs = np.abs(bw).reshape(C, -1).mean(-1)
    sw = np.exp2(np.round(np.log2(mean_abs))).astype(np.float32)
    return np.sign(bw).astype(np.float32), sw


def kernel(x, w, lut):
    x = np.ascontiguousarray(np.asarray(x, np.float32))
    w = np.asarray(w, np.float32)
    lut = np.asarray(lut, np.float32)

    nc = _build()
    from concourse import bass_utils

    # ---- weights: binarize + fold the pow2 scale into the thresholds ----
    sgn, sw = _binarize_weights(w)                     # sgn [Cout,Cin,3,3]
    t64 = lut.astype(np.float64) / sw[:, None]         # [Cout,7] thresholds

    # lhsT per tap: wts[ci, t, co] = sgn[co, ci, dh, dw]; rows 64-127 (the lo
    # half of K) use the same weights
    wts = np.empty((128, NTAPS, Cout), np.float32)
    for t in range(NTAPS):
        wts[:Cin, t, :] = sgn[:, :, t // 3, t % 3].T
    wts[Cin:] = wts[:Cin]
    wts = wts.astype(np.float16)

    # ---- normalize params: z = y*s + b with tau3 -> 0, tau5 -> 1 ----
    # s>0 always; for degenerate channels (t5 == t3) use a huge power of two
    # so [z > 1] still decides [y > t3] exactly.
    t3, t5 = t64[:, 3], t64[:, 5]
    gap = t5 - t3
    s = np.where(gap > 0, 1.0 / np.where(gap > 0, gap, 1.0), 2.0 ** 100)
    bias = -t3 * s
    taus = (t64[:, [0, 1, 2, 4, 6]] - t3[:, None]) * s[:, None]
    half = np.stack([s, bias, taus[:, 0], taus[:, 1], taus[:, 2],
                     taus[:, 3], taus[:, 4]], axis=1).astype(np.float32)
    nrm = np.empty((128, 7), np.float32)
    nrm[:Cout] = half
    nrm[Cout:] = half

    # ---- hi/lo fp16 split, zero-padded flat slabs ----
    hi = x.astype(np.float16)
    lo = (x - hi.astype(np.float32)).astype(np.float16)
    xin = np.zeros((B, 128, SLABF), np.float16)
    view = xin.reshape(B, 128, HP, WPAD)
    view[:, :Cin, 1:H + 1, 1:W + 1] = hi
    view[:, Cin:, 1:H + 1, 1:W + 1] = lo

    # ---- run on the 8 cores (SPMD, batch-sharded) ----
    wts_np = np.ascontiguousarray(wts)
    nrm_np = np.ascontiguousarray(nrm)
    in_maps = [
        {
            "xin": np.ascontiguousarray(xin[c * BPC:(c + 1) * BPC]),
            "wts": wts_np,
            "nrm": nrm_np,
        }
        for c in range(NCORES)
    ]
    try:
        res = bass_utils.run_bass_kernel_spmd(nc, in_maps,
                                              core_ids=list(range(NCORES)))
    except Exception:
        # transient PJRT/compile hiccups happen occasionally; retry once
        res = bass_utils.run_bass_kernel_spmd(nc, in_maps,
                                              core_ids=list(range(NCORES)))
    global last_results
    last_results = res

    # ---- unshard: interleave the adjacent 4-row chunks ----
    out = np.empty((B, Cout, H, W), np.float32)
    for c in range(NCORES):
        o = res.results[c]["out"]                      # [BPC, 128, HALF] u8
        top = o[:, :Cout, :].reshape(BPC, Cout, NPAIR, 4, W)
        bot = o[:, Cout:, :].reshape(BPC, Cout, NPAIR, 4, W)
        yrows = np.stack([top, bot], axis=3)           # [., ., 14, 2, 4, W]
        out[c * BPC:(c + 1) * BPC] = (
            yrows.reshape(BPC, Cout, H, W).astype(np.float32))
    return out



# revision 7
# speedup vs baseline: 1.2687x; 1.2687x over previous
"""Trainium2 Bass kernel for nn_BasicBlock_1w4a_LUT (binarized 3x3 conv + LUT bucketize).

Data-parallel over batch: 8 NeuronCores x 4 images each; no cross-core
communication. Full inputs in, full output out; shard/unshard on the host.

v2: 4 concurrent PE streams via 64x64 array tiling (T0/T2/T8/T10), fp16
hi-only activations (K=64).

Host prep:
  - Binarize the weights exactly as the reference does; the pow2 scale sw is
    folded into the LUT thresholds so device weights are exactly +/-1.
  - x is rounded to fp16 ("hi"). The dropped residual perturbs the conv
    output by sigma ~ 5e-3 against threshold spacing ~1, measured rel err
    ~4.9e-3 on the bucketized output (gate is 2e-2). PE multiplies fp16
    exactly and accumulates fp32.
  - Each 112-row image is split into 4 bands of 28 rows, one per PE tile.
    Bands A,B live in SBUF partitions 0-63 (padded rows 0..57), bands C,D
    in partitions 64-127 (padded rows 56..113); 64 channels per partition
    group, 2-row halo duplicated.
  - Per-channel affine z = y*s + b chosen so bucketize thresholds map
    tau3 -> 0 and tau5 -> 1 (frees DVE scalar slots).

Device, per image:
  - 7 quad-steps of 4-row chunks: per step, each of the 4 64x64 PE tiles
    accumulates 9 taps x 448 columns into its PSUM half-bank; the 4 streams
    run concurrently (1 col/cycle/tile at 2.4 GHz).
  - ScalarE applies the per-channel affine out of PSUM; two custom DVE ops
    compute out = sum_k [z > tau_k] over all 7 thresholds in 2 passes over
    [128, 896] chunk-pairs, writing u8.
  - Junk matmuls at kernel start warm the PE HAM clock gate (1.2 -> 2.4 GHz)
    while the first input DMA pieces land; input/output DMAs are split so
    compute starts/finishes without waiting for whole images.
"""

import numpy as np

# ---- problem constants (hardcoded per contract) ----
B, Cin, Cout, H, W = 32, 64, 64, 112, 112
NCORES = 8
BPC = B // NCORES          # images per core
HP = H + 2                 # 114 padded rows
WPAD = W + 2               # 114 padded cols
BAND = 28                  # rows per band; 4 bands per image
NB = 7                     # 4-row chunks per band
NCH = 4 * W                # 448 pixels per chunk (PSUM half-bank <= 2KB)
SLABR = 58                 # padded rows per slab half (halo of 2 shared)
SLABF = SLABR * WPAD       # 6612 fp16 per partition per image
HALFPX = BAND * W          # 3136 pixels per band
OUTF = 2 * HALFPX          # 6272 output pixels per partition per image
NTAPS = 9
NWARM = 13                 # PE warm-up matmuls

_built = []
last_results = None


def _register_dve_ops():
    from concourse.dve_spec import (
        Spec, Src0, Src1, C0, C1, C3, One, Zero, lower,
        _spill_c3_to_src1, _has_src1,
    )
    import concourse.dve_ops as dve_ops
    from concourse.dve_ops import DveOp
    from concourse.dve_uop import DveOpSpec

    def register_op(name, spec):
        if name in dve_ops._SUB_OPCODE_FOR_NAME:
            for op in dve_ops.OPS:
                if op.name == name:
                    return op
            raise RuntimeError(name)
        row = max(dve_ops._SUB_OPCODE_FOR_NAME.values()) + 1
        assert row < 0x20, "custom-DVE opcode rows exhausted"
        shas = {}
        for ver in ("v3", "v4"):
            s = DveOpSpec(name=name, opcode=row, uops=lower(spec, ver=ver),
                          rd1_en=_has_src1(spec))
            shas[ver] = s.sha(ver)
        op = DveOp(name, spec, subdim=False, uops_sha=shas)
        dve_ops.OPS.append(op)
        dve_ops.CUSTOM_DVE_SPECS[name] = spec
        dve_ops._SUB_OPCODE_FOR_NAME[name] = row
        return op

    # u = (z>tau0) + (z>tau1) + (z>tau2);  tau2 rides C3 (spilled to in1 [P,1])
    bucket3 = register_op(
        "BUCKET3_ANT",
        Spec(
            body=_spill_c3_to_src1(((Src0 > C0) + (Src0 > C1)) + (Src0 > C3)),
            reference=lambda in0, in1, s0, s1, imm2: (
                (in0 > s0).astype(np.float32) + (in0 > s1)
                + (in0 > np.asarray(in1, np.float32).reshape(-1, 1))
            ),
        ),
    )
    # out = (z>0) + (z>1) + (z>tau4) + (z>tau6) + u
    bucket4acc = register_op(
        "BUCKET4ACC_ANT",
        Spec(
            body=(((Src0 > Zero) + (Src0 > One))
                  + ((Src0 > C0) + (Src0 > C1))) + Src1,
            reference=lambda in0, in1, s0, s1, imm2: (
                (in0 > 0).astype(np.float32) + (in0 > 1)
                + (in0 > s0) + (in0 > s1) + in1
            ),
        ),
    )
    return bucket3, bucket4acc


def _build():
    """Trace + compile the per-core Bass kernel (once per process)."""
    if _built:
        return _built[0]

    import concourse.bacc as bacc
    import concourse.mybir as mybir
    import concourse.tile as tile

    bucket3, bucket4acc = _register_dve_ops()

    f32, f16, u8 = mybir.dt.float32, mybir.dt.float16, mybir.dt.uint8
    nc = bacc.Bacc("TRN2", target_bir_lowering=False, debug=False,
                   num_devices=NCORES)

    xin_t = nc.dram_tensor("xin", [BPC, 128, SLABF], f16, kind="ExternalInput")
    wts_t = nc.dram_tensor("wts", [128, NTAPS, Cout], f16, kind="ExternalInput")
    nrm_t = nc.dram_tensor("nrm", [128, 7], f32, kind="ExternalInput")
    out_t = nc.dram_tensor("out", [BPC, 128, OUTF], u8, kind="ExternalOutput")

    with tile.TileContext(nc) as tc:
        with (
            tc.tile_pool(name="const", bufs=1) as cpool,
            tc.tile_pool(name="slab", bufs=2) as spool,
            tc.tile_pool(name="psum", bufs=8, space="PSUM") as ppool,
            tc.tile_pool(name="z", bufs=4) as zpool,
            tc.tile_pool(name="u", bufs=4) as upool,
            tc.tile_pool(name="o", bufs=2) as opool,
        ):
            # first slab piece of image 0 goes out before anything else — it
            # gates the first quad-step (needs slab rows 0..33 = cols 0:3900)
            slab0 = spool.tile([128, SLABF], f16, tag="slab")
            nc.sync.dma_start(out=slab0[:, 0:3900], in_=xin_t.ap()[0, :, 0:3900])
            wts = cpool.tile([128, NTAPS, Cout], f16)
            nc.scalar.dma_start(out=wts[:], in_=wts_t.ap())
            nrm = cpool.tile([128, 7], f32)
            nc.scalar.dma_start(out=nrm[:], in_=nrm_t.ap())

            # PE warm-up: junk matmuls on a zeroed tile while the first input
            # DMAs land, so the HAM clock gate opens (1.2 -> 2.4 GHz) before
            # the real matmuls start. Must use the same 64x64 tiling mode as
            # the real matmuls (mode switches drain the PE).
            wu = cpool.tile([128, NCH], f16)
            nc.gpsimd.memset(wu[:], 0.0)
            # only the two column tiles of row-group 0: row tiles must not
            # hit the same PSUM bank concurrently
            wps = ppool.tile([128, NCH], f32, name="ps", tag="ps")
            for i in range(NWARM):
                tp = ((0, 0), (0, 64))[i % 2]
                nc.tensor.matmul(wps[tp[1]:tp[1] + 64, :],
                                 wu[0:64, 0:Cout], wu[0:64, :],
                                 tile_position=tp, start=True, stop=True)

            scale, bias = nrm[:, 0:1], nrm[:, 1:2]
            tau0, tau1, tau2 = nrm[:, 2:3], nrm[:, 3:4], nrm[:, 4:5]
            tau4, tau6 = nrm[:, 5:6], nrm[:, 6:7]

            for b in range(BPC):
                # split the input DMA so early chunks can start sooner
                if b == 0:
                    slab = slab0
                    cuts = [3900, 5256, SLABF]
                else:
                    slab = spool.tile([128, SLABF], f16, tag="slab")
                    cuts = [2204, 4408, SLABF]
                lo = 0
                for n, hi in enumerate(cuts):
                    if b == 0 and n == 0:
                        lo = hi
                        continue
                    eng = nc.sync if n % 2 == 0 else nc.scalar
                    eng.dma_start(out=slab[:, lo:hi], in_=xin_t.ap()[b, :, lo:hi])
                    lo = hi
                slabv = slab[:].rearrange("p (r w) -> p r w", w=WPAD)

                oslab = opool.tile([128, OUTF], u8)
                # quad-pair groups: chunks (0,1),(2,3),(4,5),(6,)
                for g in range(4):
                    chunks = (2 * g, 2 * g + 1) if g < 3 else (6,)
                    nch = NCH * len(chunks)
                    psA = {}
                    psB = {}
                    for c in chunks:
                        psA[c] = ppool.tile([128, NCH], f32, name="ps",
                                            tag="ps")
                        psB[c] = ppool.tile([128, NCH], f32, name="ps",
                                            tag="ps")
                    # 4 concurrent streams: T0=(0,0) band A, T2=(0,64) band B,
                    # T8=(64,0) band C, T10=(64,64) band D. Bands A/C share
                    # chunk-local row indices (their slabs are offset copies).
                    for t in range(NTAPS):
                        dh, dw = divmod(t, 3)
                        st, sp = (t == 0), (t == NTAPS - 1)
                        for c in chunks:
                            rlo = 4 * c + dh
                            rhi = BAND + rlo
                            nc.tensor.matmul(
                                psA[c][0:64, :], wts[0:64, t, :],
                                slabv[0:64, rlo:rlo + 4, dw:dw + W],
                                tile_position=(0, 0), start=st, stop=sp)
                            nc.tensor.matmul(
                                psA[c][64:128, :], wts[0:64, t, :],
                                slabv[0:64, rhi:rhi + 4, dw:dw + W],
                                tile_position=(0, 64), start=st, stop=sp)
                            nc.tensor.matmul(
                                psB[c][0:64, :], wts[64:128, t, :],
                                slabv[64:128, rlo:rlo + 4, dw:dw + W],
                                tile_position=(64, 0), start=st, stop=sp)
                            nc.tensor.matmul(
                                psB[c][64:128, :], wts[64:128, t, :],
                                slabv[64:128, rhi:rhi + 4, dw:dw + W],
                                tile_position=(64, 64), start=st, stop=sp)

                    # evacuate: scalar affine PSUM->SBUF, then 2 DVE passes
                    # over the [128, nch] group, u8 straight into the oslab
                    for half, ps in (("AB", psA), ("CD", psB)):
                        z = zpool.tile([128, nch], f32)
                        for k, c in enumerate(chunks):
                            nc.scalar.activation(
                                z[:, k * NCH:(k + 1) * NCH], ps[c][:],
                                mybir.ActivationFunctionType.Identity,
                                bias=bias, scale=scale)
                        u = upool.tile([128, nch], mybir.dt.bfloat16)
                        nc.vector._custom_dve(
                            bucket3, out=u[:], in0=z[:],
                            in1=tau2, s0=tau0, s1=tau1)
                        col = (0 if half == "AB" else HALFPX) + chunks[0] * NCH
                        nc.vector._custom_dve(
                            bucket4acc, out=oslab[:, col:col + nch],
                            in0=z[:], in1=u[:], s0=tau4, s1=tau6)

                # split output DMA so early pieces leave while later chunks
                # are still being computed; finest for the last image (tail)
                if b == BPC - 1:
                    ocuts = [0, HALFPX, HALFPX + 1792, OUTF - 896, OUTF]
                else:
                    ocuts = [0, HALFPX, OUTF]
                for lo, hi in zip(ocuts[:-1], ocuts[1:]):
                    nc.sync.dma_start(out=out_t.ap()[b, :, lo:hi],
                                      in_=oslab[:, lo:hi])

    nc.compile()
    _built.append(nc)
    return nc


def _binarize_weights(w):
    """Exactly the reference's fp32 binarization. Returns (sign in {-1,0,1}, sw)."""
    w = np.asarray(w, np.float32)
    C = w.shape[0]
    wf = w.reshape(C, -1)
    bw = w - wf.mean(-1)[:, None, None, None]
    bw = bw / bw.reshape(C, -1).std(-1, ddof=1)[:, None, None, None]
    mean_abs = np.abs(bw).reshape(C, -1).mean(-1)
    sw = np.exp2(np.round(np.log2(mean_abs))).astype(np.float32)
    return np.sign(bw).astype(np.float32), sw


def kernel(x, w, lut):
    x = np.ascontiguousarray(np.asarray(x, np.float32))
    w = np.asarray(w, np.float32)
    lut = np.asarray(lut, np.float32)

    nc = _build()
    from concourse import bass_utils

    # ---- weights: binarize + fold the pow2 scale into the thresholds ----
    sgn, sw = _binarize_weights(w)                     # sgn [Cout,Cin,3,3]
    t64 = lut.astype(np.float64) / sw[:, None]         # [Cout,7] thresholds

    # lhsT per tap: wts[ci, t, co] = sgn[co, ci, dh, dw]; rows 64-127 serve
    # the row-tiled PE tiles T8/T10 (same weights, SBUF partitions 64-127)
    wts = np.empty((128, NTAPS, Cout), np.float32)
    for t in range(NTAPS):
        wts[:Cin, t, :] = sgn[:, :, t // 3, t % 3].T
    wts[Cin:] = wts[:Cin]
    wts = wts.astype(np.float16)

    # ---- normalize params: z = y*s + b with tau3 -> 0, tau5 -> 1 ----
    # s>0 always; for degenerate channels (t5 == t3) use a huge power of two
    # so [z > 1] still decides [y > t3] exactly.
    t3, t5 = t64[:, 3], t64[:, 5]
    gap = t5 - t3
    s = np.where(gap > 0, 1.0 / np.where(gap > 0, gap, 1.0), 2.0 ** 100)
    bias = -t3 * s
    taus = (t64[:, [0, 1, 2, 4, 6]] - t3[:, None]) * s[:, None]
    half = np.stack([s, bias, taus[:, 0], taus[:, 1], taus[:, 2],
                     taus[:, 3], taus[:, 4]], axis=1).astype(np.float32)
    nrm = np.empty((128, 7), np.float32)
    nrm[:Cout] = half
    nrm[Cout:] = half

    # ---- fp16 slabs: bands A,B (padded rows 0..57) in partitions 0-63,
    # bands C,D (padded rows 56..113) in partitions 64-127 ----
    hi16 = x.astype(np.float16)
    xin = np.zeros((B, 128, SLABF), np.float16)
    view = xin.reshape(B, 128, SLABR, WPAD)
    view[:, :Cin, 1:58, 1:W + 1] = hi16[:, :, 0:57, :]
    view[:, Cin:, 0:57, 1:W + 1] = hi16[:, :, 55:112, :]

    # ---- run on the 8 cores (SPMD, batch-sharded) ----
    wts_np = np.ascontiguousarray(wts)
    nrm_np = np.ascontiguousarray(nrm)
    in_maps = [
        {
            "xin": np.ascontiguousarray(xin[c * BPC:(c + 1) * BPC]),
            "wts": wts_np,
            "nrm": nrm_np,
        }
        for c in range(NCORES)
    ]
    try:
        res = bass_utils.run_bass_kernel_spmd(nc, in_maps,
                                              core_ids=list(range(NCORES)))
    except Exception:
        # transient PJRT/compile hiccups happen occasionally; retry once
        res = bass_utils.run_bass_kernel_spmd(nc, in_maps,
                                              core_ids=list(range(NCORES)))
    global last_results
    last_results = res

    # ---- unshard: cols 0:3136 = bands A (part 0-63) / B (64-127),
    # cols 3136:6272 = bands C / D ----
    out = np.empty((B, Cout, H, W), np.float32)
    for c in range(NCORES):
        o = res.results[c]["out"]                      # [BPC, 128, OUTF] u8
        ab = o[:, :, :HALFPX].reshape(BPC, 2, Cout, NB, 4, W)
        cd = o[:, :, HALFPX:].reshape(BPC, 2, Cout, NB, 4, W)
        bands = np.stack([ab[:, 0], ab[:, 1], cd[:, 0], cd[:, 1]], axis=1)
        out[c * BPC:(c + 1) * BPC] = (
            bands.transpose(0, 2, 1, 3, 4, 5)
            .reshape(BPC, Cout, H, W).astype(np.float32))
    return out


# revision 8
# speedup vs baseline: 1.3098x; 1.0324x over previous
"""Trainium2 Bass kernel for nn_BasicBlock_1w4a_LUT (binarized 3x3 conv + LUT bucketize).

Data-parallel over batch: 8 NeuronCores x 4 images each; no cross-core
communication. Full inputs in, full output out; shard/unshard on the host.

v3: 4 concurrent PE streams via 64x64 array tiling (T0/T2/T8/T10), fp16
hi-only activations (K=64), 2-bank PSUM tiles, batched DVE.

Host prep:
  - Binarize the weights exactly as the reference does; the pow2 scale sw is
    folded into the LUT thresholds so device weights are exactly +/-1.
  - x is rounded to fp16 ("hi"). The dropped residual perturbs the conv
    output by sigma ~ 5e-3 against threshold spacing ~1, measured rel err
    ~5e-3 on the bucketized output (gate is 2e-2). PE multiplies fp16
    exactly and accumulates fp32.
  - Each 112-row image is split into 4 bands of 28 rows, one per PE tile.
    Bands A,B live in SBUF partitions 0-63 (padded rows 0..57), bands C,D
    in partitions 64-127 (padded rows 56..113); 64 channels per partition
    group, 2-row halo duplicated.
  - Per-channel affine z = y*s + b chosen so bucketize thresholds map
    tau3 -> 0 and tau5 -> 1 (frees DVE scalar slots).

Device, per image:
  - 7 quad-steps of 4-row chunks: per step, each of the 4 64x64 PE tiles
    accumulates 9 taps x 448 columns into its PSUM half-bank; the 4 streams
    run concurrently (1 col/cycle/tile at 2.4 GHz). PSUM tiles span 2 banks
    ([128, 2, 448] padded to 512) so one chunk-pair evacuates per scalar op.
  - ScalarE applies the per-channel affine out of PSUM; two custom DVE ops
    compute out = sum_k [z > tau_k] over all 7 thresholds in 2 passes over
    [128, 1792/1344] batches, writing u8.
  - PE warm-up matmuls run on the weights tile itself (no memset
    dependency) so the HAM clock gate opens (1.2 -> 2.4 GHz) while the
    first input slab lands.
"""

import numpy as np

# ---- problem constants (hardcoded per contract) ----
B, Cin, Cout, H, W = 32, 64, 64, 112, 112
NCORES = 8
BPC = B // NCORES          # images per core
HP = H + 2                 # 114 padded rows
WPAD = H + 2               # 114 padded cols
BAND = 28                  # rows per band; 4 bands per image
NB = 7                     # 4-row chunks per band
NCH = 4 * W                # 448 pixels per chunk (one PSUM bank)
SLABR = 58                 # padded rows per slab half (halo of 2 shared)
SLABF = SLABR * WPAD       # 6612 fp16 per partition per image
HALFPX = BAND * W          # 3136 pixels per band
OUTF = 2 * HALFPX          # 6272 output pixels per partition per image
NTAPS = 9
NWARM = 10                 # PE warm-up matmuls

_built = []
last_results = None


def _register_dve_ops():
    from concourse.dve_spec import (
        Spec, Src0, Src1, C0, C1, C3, One, Zero, lower,
        _spill_c3_to_src1, _has_src1,
    )
    import concourse.dve_ops as dve_ops
    from concourse.dve_ops import DveOp
    from concourse.dve_uop import DveOpSpec

    def register_op(name, spec):
        if name in dve_ops._SUB_OPCODE_FOR_NAME:
            for op in dve_ops.OPS:
                if op.name == name:
                    return op
            raise RuntimeError(name)
        row = max(dve_ops._SUB_OPCODE_FOR_NAME.values()) + 1
        assert row < 0x20, "custom-DVE opcode rows exhausted"
        shas = {}
        for ver in ("v3", "v4"):
            s = DveOpSpec(name=name, opcode=row, uops=lower(spec, ver=ver),
                          rd1_en=_has_src1(spec))
            shas[ver] = s.sha(ver)
        op = DveOp(name, spec, subdim=False, uops_sha=shas)
        dve_ops.OPS.append(op)
        dve_ops.CUSTOM_DVE_SPECS[name] = spec
        dve_ops._SUB_OPCODE_FOR_NAME[name] = row
        return op

    # u = (z>tau0) + (z>tau1) + (z>tau2);  tau2 rides C3 (spilled to in1 [P,1])
    bucket3 = register_op(
        "BUCKET3_ANT",
        Spec(
            body=_spill_c3_to_src1(((Src0 > C0) + (Src0 > C1)) + (Src0 > C3)),
            reference=lambda in0, in1, s0, s1, imm2: (
                (in0 > s0).astype(np.float32) + (in0 > s1)
                + (in0 > np.asarray(in1, np.float32).reshape(-1, 1))
            ),
        ),
    )
    # out = (z>0) + (z>1) + (z>tau4) + (z>tau6) + u
    bucket4acc = register_op(
        "BUCKET4ACC_ANT",
        Spec(
            body=(((Src0 > Zero) + (Src0 > One))
                  + ((Src0 > C0) + (Src0 > C1))) + Src1,
            reference=lambda in0, in1, s0, s1, imm2: (
                (in0 > 0).astype(np.float32) + (in0 > 1)
                + (in0 > s0) + (in0 > s1) + in1
            ),
        ),
    )
    return bucket3, bucket4acc


def _build():
    """Trace + compile the per-core Bass kernel (once per process)."""
    if _built:
        return _built[0]

    import concourse.bacc as bacc
    import concourse.mybir as mybir
    import concourse.tile as tile

    bucket3, bucket4acc = _register_dve_ops()

    f32, f16, u8 = mybir.dt.float32, mybir.dt.float16, mybir.dt.uint8
    nc = bacc.Bacc("TRN2", target_bir_lowering=False, debug=False,
                   num_devices=NCORES)

    xin_t = nc.dram_tensor("xin", [BPC, 128, SLABF], f16, kind="ExternalInput")
    wts_t = nc.dram_tensor("wts", [128, NTAPS, Cout], f16, kind="ExternalInput")
    nrm_t = nc.dram_tensor("nrm", [128, 7], f32, kind="ExternalInput")
    out_t = nc.dram_tensor("out", [BPC, 128, OUTF], u8, kind="ExternalOutput")

    with tile.TileContext(nc) as tc:
        with (
            tc.tile_pool(name="const", bufs=1) as cpool,
            tc.tile_pool(name="slab", bufs=2) as spool,
            tc.tile_pool(name="psum", bufs=4, space="PSUM") as ppool,
            tc.tile_pool(name="z", bufs=4) as zpool,
            tc.tile_pool(name="u", bufs=4) as upool,
            tc.tile_pool(name="o", bufs=2) as opool,
        ):
            # weights first (small) — they gate the PE warm-up
            wts = cpool.tile([128, NTAPS, Cout], f16)
            nc.sync.dma_start(out=wts[:], in_=wts_t.ap())
            nrm = cpool.tile([128, 7], f32)
            nc.scalar.dma_start(out=nrm[:], in_=nrm_t.ap())
            # first slab piece of image 0 (first quad-step needs rows 0..33)
            slab0 = spool.tile([128, SLABF], f16, tag="slab")
            nc.sync.dma_start(out=slab0[:, 0:3900], in_=xin_t.ap()[0, :, 0:3900])

            # PE warm-up: junk matmuls on the weights tile itself while the
            # first input DMA lands, so the HAM clock gate opens
            # (1.2 -> 2.4 GHz) before the real matmuls start. Same 64x64
            # tiling mode as the real matmuls (mode switches drain the PE);
            # only the two column tiles of row-group 0 (row tiles must not
            # hit the same PSUM bank concurrently).
            wps = ppool.tile([128, 2, NCH], f32, name="ps", tag="ps",
                             padded_shape=[128, 2, 512])
            for i in range(NWARM):
                tp = ((0, 0), (0, 64))[i % 2]
                nc.tensor.matmul(wps[tp[1]:tp[1] + 64, i % 2, :],
                                 wts[0:64, 0, :], wts[0:64, 0:7, :],
                                 tile_position=tp, start=True, stop=True)

            scale, bias = nrm[:, 0:1], nrm[:, 1:2]
            tau0, tau1, tau2 = nrm[:, 2:3], nrm[:, 3:4], nrm[:, 4:5]
            tau4, tau6 = nrm[:, 5:6], nrm[:, 6:7]

            for b in range(BPC):
                # split the input DMA so early chunks can start sooner
                if b == 0:
                    slab = slab0
                    cuts = [3900, 5256, SLABF]
                else:
                    slab = spool.tile([128, SLABF], f16, tag="slab")
                    cuts = [2204, 4408, SLABF]
                lo = 0
                for n, hi in enumerate(cuts):
                    if b == 0 and n == 0:
                        lo = hi
                        continue
                    eng = nc.sync if n % 2 == 0 else nc.scalar
                    eng.dma_start(out=slab[:, lo:hi], in_=xin_t.ap()[b, :, lo:hi])
                    lo = hi
                slabv = slab[:].rearrange("p (r w) -> p r w", w=WPAD)

                oslab = opool.tile([128, OUTF], u8)
                # DVE batches: chunk groups (0,1) -> 1792 cols, (2,3) with
                # the odd 7th chunk -> 1344 cols
                for batch, (glist, bcols) in enumerate(
                        (((0, 1), 1792), ((2, 3), 1344))):
                    zA = zpool.tile([128, bcols], f32, name="zA", tag="z")
                    zB = zpool.tile([128, bcols], f32, name="zB", tag="z")
                    zoff = 0
                    for g in glist:
                        chunks = (2 * g, 2 * g + 1) if g < 3 else (6,)
                        ncols = NCH * len(chunks)
                        psA = ppool.tile([128, 2, NCH], f32, name="ps",
                                         tag="ps", padded_shape=[128, 2, 512])
                        psB = ppool.tile([128, 2, NCH], f32, name="ps",
                                         tag="ps", padded_shape=[128, 2, 512])
                        # 4 concurrent streams: T0=(0,0) band A, T2=(0,64)
                        # band B, T8=(64,0) band C, T10=(64,64) band D.
                        for t in range(NTAPS):
                            dh, dw = divmod(t, 3)
                            st, sp = (t == 0), (t == NTAPS - 1)
                            for k, c in enumerate(chunks):
                                rlo = 4 * c + dh
                                rhi = BAND + rlo
                                nc.tensor.matmul(
                                    psA[0:64, k, :], wts[0:64, t, :],
                                    slabv[0:64, rlo:rlo + 4, dw:dw + W],
                                    tile_position=(0, 0), start=st, stop=sp)
                                nc.tensor.matmul(
                                    psA[64:128, k, :], wts[0:64, t, :],
                                    slabv[0:64, rhi:rhi + 4, dw:dw + W],
                                    tile_position=(0, 64), start=st, stop=sp)
                                nc.tensor.matmul(
                                    psB[0:64, k, :], wts[64:128, t, :],
                                    slabv[64:128, rlo:rlo + 4, dw:dw + W],
                                    tile_position=(64, 0), start=st, stop=sp)
                                nc.tensor.matmul(
                                    psB[64:128, k, :], wts[64:128, t, :],
                                    slabv[64:128, rhi:rhi + 4, dw:dw + W],
                                    tile_position=(64, 64), start=st, stop=sp)
                        # evacuate the 2-bank tile with one scalar op each
                        for z, ps in ((zA, psA), (zB, psB)):
                            nc.scalar.activation(
                                z[:, zoff:zoff + ncols].rearrange(
                                    "p (k c) -> p k c", c=NCH),
                                ps[:, 0:len(chunks), :],
                                mybir.ActivationFunctionType.Identity,
                                bias=bias, scale=scale)
                        zoff += ncols

                    # 2 DVE passes per batch per band-pair, u8 into oslab
                    for z, base in ((zA, 0), (zB, HALFPX)):
                        u = upool.tile([128, bcols], mybir.dt.bfloat16)
                        nc.vector._custom_dve(
                            bucket3, out=u[:], in0=z[:],
                            in1=tau2, s0=tau0, s1=tau1)
                        col = base + batch * 1792
                        nc.vector._custom_dve(
                            bucket4acc, out=oslab[:, col:col + bcols],
                            in0=z[:], in1=u[:], s0=tau4, s1=tau6)

                # split output DMA so early pieces leave while later chunks
                # are still being computed; finest for the last image (tail)
                if b == BPC - 1:
                    ocuts = [0, HALFPX, HALFPX + 1792, OUTF - 896, OUTF]
                else:
                    ocuts = [0, HALFPX, OUTF]
                for lo, hi in zip(ocuts[:-1], ocuts[1:]):
                    nc.sync.dma_start(out=out_t.ap()[b, :, lo:hi],
                                      in_=oslab[:, lo:hi])

    nc.compile()
    _built.append(nc)
    return nc


def _binarize_weights(w):
    """Exactly the reference's fp32 binarization. Returns (sign in {-1,0,1}, sw)."""
    w = np.asarray(w, np.float32)
    C = w.shape[0]
    wf = w.reshape(C, -1)
    bw = w - wf.mean(-1)[:, None, None, None]
    bw = bw / bw.reshape(C, -1).std(-1, ddof=1)[:, None, None, None]
    mean_abs = np.abs(bw).reshape(C, -1).mean(-1)
    sw = np.exp2(np.round(np.log2(mean_abs))).astype(np.float32)
    return np.sign(bw).astype(np.float32), sw


def kernel(x, w, lut):
    x = np.ascontiguousarray(np.asarray(x, np.float32))
    w = np.asarray(w, np.float32)
    lut = np.asarray(lut, np.float32)

    nc = _build()
    from concourse import bass_utils

    # ---- weights: binarize + fold the pow2 scale into the thresholds ----
    sgn, sw = _binarize_weights(w)                     # sgn [Cout,Cin,3,3]
    t64 = lut.astype(np.float64) / sw[:, None]         # [Cout,7] thresholds

    # lhsT per tap: wts[ci, t, co] = sgn[co, ci, dh, dw]; rows 64-127 serve
    # the row-tiled PE tiles T8/T10 (same weights, SBUF partitions 64-127)
    wts = np.empty((128, NTAPS, Cout), np.float32)
    for t in range(NTAPS):
        wts[:Cin, t, :] = sgn[:, :, t // 3, t % 3].T
    wts[Cin:] = wts[:Cin]
    wts = wts.astype(np.float16)

    # ---- normalize params: z = y*s + b with tau3 -> 0, tau5 -> 1 ----
    # s>0 always; for degenerate channels (t5 == t3) use a huge power of two
    # so [z > 1] still decides [y > t3] exactly.
    t3, t5 = t64[:, 3], t64[:, 5]
    gap = t5 - t3
    s = np.where(gap > 0, 1.0 / np.where(gap > 0, gap, 1.0), 2.0 ** 100)
    bias = -t3 * s
    taus = (t64[:, [0, 1, 2, 4, 6]] - t3[:, None]) * s[:, None]
    half = np.stack([s, bias, taus[:, 0], taus[:, 1], taus[:, 2],
                     taus[:, 3], taus[:, 4]], axis=1).astype(np.float32)
    nrm = np.empty((128, 7), np.float32)
    nrm[:Cout] = half
    nrm[Cout:] = half

    # ---- fp16 slabs: bands A,B (padded rows 0..57) in partitions 0-63,
    # bands C,D (padded rows 56..113) in partitions 64-127 ----
    hi16 = x.astype(np.float16)
    xin = np.zeros((B, 128, SLABF), np.float16)
    view = xin.reshape(B, 128, SLABR, WPAD)
    view[:, :Cin, 1:58, 1:W + 1] = hi16[:, :, 0:57, :]
    view[:, Cin:, 0:57, 1:W + 1] = hi16[:, :, 55:112, :]

    # ---- run on the 8 cores (SPMD, batch-sharded) ----
    wts_np = np.ascontiguousarray(wts)
    nrm_np = np.ascontiguousarray(nrm)
    in_maps = [
        {
            "xin": np.ascontiguousarray(xin[c * BPC:(c + 1) * BPC]),
            "wts": wts_np,
            "nrm": nrm_np,
        }
        for c in range(NCORES)
    ]
    try:
        res = bass_utils.run_bass_kernel_spmd(nc, in_maps,
                                              core_ids=list(range(NCORES)))
    except Exception:
        # transient PJRT/compile hiccups happen occasionally; retry once
        res = bass_utils.run_bass_kernel_spmd(nc, in_maps,
                                              core_ids=list(range(NCORES)))
    global last_results
    last_results = res

    # ---- unshard: cols 0:3136 = bands A (part 0-63) / B (64-127),
    # cols 3136:6272 = bands C / D ----
    out = np.empty((B, Cout, H, W), np.float32)
    for c in range(NCORES):
        o = res.results[c]["out"]                      # [BPC, 128, OUTF] u8
        ab = o[:, :, :HALFPX].reshape(BPC, 2, Cout, NB, 4, W)
        cd = o[:, :, HALFPX:].reshape(BPC, 2, Cout, NB, 4, W)
        bands = np.stack([ab[:, 0], ab[:, 1], cd[:, 0], cd[:, 1]], axis=1)
        out[c * BPC:(c + 1) * BPC] = (
            bands.transpose(0, 2, 1, 3, 4, 5)
            .reshape(BPC, Cout, H, W).astype(np.float32))
    return out
